# revision 1
# baseline (speedup 1.0000x reference)
"""Trainium2 Bass kernel for nn_Attention_28338194219036.

GQA attention block (QKV proj + QK-RMSNorm + RoPE + causal SDPA + out proj)
for x:[2,2048,2048], 16 q-heads / 4 kv-heads, head_dim 128.

Distribution over 8 NeuronCores: 2-way data parallel on batch x 4-way tensor
parallel on heads. Core c handles batch b=c//4 and TP rank r=c%4 (q-heads
4r..4r+3, kv-head r). Attention runs per 512-token query chunk; after each
chunk a 4-rank AllGather exchanges that chunk's head-shards of y^T, and an
output-projection pass for that chunk starts immediately (each core computes
its own 512 OUTPUT CHANNELS of Wo — selected with a partition-id based
dynamic DMA offset into Wo — for all tokens), so communication and the out
projection overlap the remaining attention chunks. The host concatenates the
channel slices.

All activations live transposed ([channels, tokens]) so every matmul
contraction runs over the partition axis. Matmuls run in float32r (full PE
rate at N>=256, ~1e-4 rounding). RMSNorm reduces over the partition axis via
a ones-vector matmul; RoPE's rotate-half is an SBUF->SBUF partition-swap DMA
(with a pre-swapped sin table); softmax needs no max subtraction because
QK-RMSNorm bounds |scores|*scale by sqrt(128). The four per-head softmax
denominators pack into one PSUM tile via col-tiling (tile_position) and run
concurrently on the PE. Diagonal score blocks compute only their valid
column suffix, with a single 128x128 additive triangle mask.
"""

import os
import sys

for _p in ("/opt/trn_rl_repo", "/root/.axon_site/_ro/trn_rl_repo"):
    if os.path.isdir(_p) and _p not in sys.path:
        sys.path.append(_p)

import numpy as np

B, T, C = 2, 2048, 2048
NH, NKV, HD = 16, 4, 128
TP = 4            # tensor-parallel group size
NCORES = 8
QH = NH // TP     # q-heads per core (4)
QD = QH * HD      # q channels per core (512)
TC = 4            # token chunks of 512
TCH = T // TC     # 512
CCH = C // 128    # 16 channel chunks
ROPE_BASE = 10000.0
SCALE = 1.0 / float(np.sqrt(HD))
EPS = float(np.finfo(np.float32).eps)
NEG = -1.0e9
REPEAT = 1
NO_COLLECTIVE = False
PHASES = 3

_CACHE = {}


def _build_nc():
    import concourse.mybir as mybir
    import concourse.tile as tile
    import concourse.bass as bass
    from concourse import bacc
    from concourse.masks import make_identity

    F32 = mybir.dt.float32
    F32R = mybir.dt.float32r
    AF = mybir.ActivationFunctionType

    nc = bacc.Bacc("TRN2", target_bir_lowering=False, debug=False, num_devices=NCORES)

    xT = nc.dram_tensor("xT", [C, T], F32, kind="ExternalInput")
    wqT = nc.dram_tensor("wqT", [C, QD], F32, kind="ExternalInput")
    wkT = nc.dram_tensor("wkT", [C, HD], F32, kind="ExternalInput")
    wvT = nc.dram_tensor("wvT", [C, HD], F32, kind="ExternalInput")
    woT = nc.dram_tensor("woT", [C, C], F32, kind="ExternalInput")
    ccT = nc.dram_tensor("ccT", [HD, T], F32, kind="ExternalInput")
    ssT = nc.dram_tensor("ssT", [HD, T], F32, kind="ExternalInput")
    masks = nc.dram_tensor("masks", [128, 128], F32, kind="ExternalInput")
    ones_in = nc.dram_tensor("ones_in", [128, 128], F32, kind="ExternalInput")
    outT = nc.dram_tensor("outT", [QD, T], F32, kind="ExternalOutput")

    with tile.TileContext(nc) as tc:
        for _rep in range(REPEAT):
            with tc.tile_pool(name="drp", bufs=1, space="DRAM") as drp:
                y_loc_t = [drp.tile([QD, TCH], F32, name=f"y_loc{t}") for t in range(TC)]
                y_all_t = [drp.tile([C, TCH], F32, name=f"y_all{t}") for t in range(TC)]

                with tc.tile_pool(name="pa", bufs=1) as pa:
                    ones_r = pa.tile([128, 128], F32R, name="ones_r")
                    nc.sync.dma_start(ones_r[:], ones_in[:].bitcast(F32R))
                    ident = pa.tile([128, 128], F32, name="ident")
                    make_identity(nc, ident[:])
                    epst = pa.tile([1, 1], F32, name="epst")
                    nc.any.memset(epst[:], EPS)
                    qhat = [pa.tile([128, T], F32R, name=f"qhat{h}") for h in range(QH)]
                    khat = pa.tile([128, T], F32R, name="khat")
                    vnat = [pa.tile([128, 128], F32R, name=f"vnat{j}") for j in range(T // 128)]

                    # ---------------- projection phase ----------------
                    with (
                        tc.tile_pool(name="pw", bufs=1) as pw,
                        tc.tile_pool(name="px", bufs=1) as px,
                        tc.tile_pool(name="psp", bufs=1, space="PSUM") as psp,
                    ):
                        # startup order: K weights + first x chunk + tables
                        # first so the PE can start within a few microseconds,
                        # then the rest of the weights.
                        wk_s = []
                        x_chunks = {}
                        for cci in range(CCH):
                            wk_t = pw.tile([128, HD], F32R, name=f"wk{cci}")
                            nc.sync.dma_start(wk_t[:], wkT[cci * 128 : cci * 128 + 128, :].bitcast(F32R))
                            wk_s.append(wk_t)
                            x_t = px.tile([128, TCH], F32R, tag="x", bufs=20, name=f"x0_{cci}")
                            nc.sync.dma_start(x_t[:], xT[cci * 128 : cci * 128 + 128, 0:TCH].bitcast(F32R))
                            x_chunks[(0, cci)] = x_t
                        cc_s = pw.tile([128, T], F32, name="cc_s")
                        ss_s = pw.tile([128, T], F32, name="ss_s")
                        nc.sync.dma_start(cc_s[:], ccT[:])
                        nc.sync.dma_start(ss_s[:], ssT[:])
                        wq_s, wv_s = [], []
                        for cci in range(CCH):
                            wq_t = pw.tile([128, QD], F32R, name=f"wq{cci}")
                            nc.sync.dma_start(wq_t[:], wqT[cci * 128 : cci * 128 + 128, :].bitcast(F32R))
                            wq_s.append(wq_t)
                            wv_t = pw.tile([128, HD], F32R, name=f"wv{cci}")
                            nc.sync.dma_start(wv_t[:], wvT[cci * 128 : cci * 128 + 128, :].bitcast(F32R))
                            wv_s.append(wv_t)

                        def norm_rope(x_ps, dest_slice, tci):
                            """RMSNorm + RoPE a [128(head dim), 512(tokens)]
                            psum chunk into dest_slice (F32R sbuf)."""
                            tsl = slice(tci * TCH, (tci + 1) * TCH)
                            sq = px.tile([128, TCH], F32R, tag="sq", bufs=2, name="sq")
                            nc.scalar.activation(sq[:], x_ps[:], AF.Square)
                            msq = psp.tile([1, TCH], F32, tag="ms", bufs=2, name="msq")
                            nc.tensor.matmul(msq[:], lhsT=ones_r[:, 0:1], rhs=sq[:], start=True, stop=True)
                            srt = px.tile([1, TCH], F32, tag="srt", bufs=2, name="srt")
                            nc.scalar.activation(srt[:], msq[:], AF.Sqrt, bias=epst[:], scale=1.0 / HD)
                            rin = px.tile([1, TCH], F32, tag="rin", bufs=2, name="rin")
                            nc.vector.reciprocal(rin[:], srt[:])
                            rbc = px.tile([128, TCH], F32, tag="rbc", bufs=2, name="rbc")
                            nc.gpsimd.partition_broadcast(rbc[:], rin[:])
                            # RoPE: xhat = x*cc + swap(x*ss_preswapped)
                            t1 = px.tile([128, TCH], F32, tag="t1", bufs=2, name="t1")
                            nc.vector.tensor_mul(t1[:], x_ps[:], ss_s[:, tsl])
                            t2 = px.tile([128, TCH], F32, tag="t2", bufs=2, name="t2")
                            nc.sync.dma_start(t2[0:64, :], t1[64:128, :])
                            nc.sync.dma_start(t2[64:128, :], t1[0:64, :])
                            u = px.tile([128, TCH], F32, tag="u", bufs=2, name="u")
                            nc.vector.tensor_mul(u[:], x_ps[:], cc_s[:, tsl])
                            v = px.tile([128, TCH], F32, tag="v", bufs=2, name="v")
                            nc.vector.tensor_add(v[:], u[:], t2[:])
                            nc.vector.tensor_mul(dest_slice, v[:], rbc[:])

                        for tci in range(TC):
                            tsl = slice(tci * TCH, (tci + 1) * TCH)
                            x_s = []
                            for cci in range(CCH):
                                if (tci, cci) in x_chunks:
                                    x_s.append(x_chunks.pop((tci, cci)))
                                    continue
                                x_t = px.tile([128, TCH], F32R, tag="x", bufs=20, name=f"x{tci}_{cci}")
                                nc.sync.dma_start(x_t[:], xT[cci * 128 : cci * 128 + 128, tsl].bitcast(F32R))
                                x_s.append(x_t)

                            # K projection -> khat
                            k_ps = psp.tile([128, TCH], F32, tag="xp", bufs=4, name="k_ps")
                            for cci in range(CCH):
                                nc.tensor.matmul(
                                    k_ps[:], lhsT=wk_s[cci][:], rhs=x_s[cci][:],
                                    start=(cci == 0), stop=(cci == CCH - 1),
                                )
                            norm_rope(k_ps, khat[:, tsl], tci)

                            # Q projections -> qhat[h]
                            for h in range(QH):
                                q_ps = psp.tile([128, TCH], F32, tag="xp", bufs=4, name="q_ps")
                                for cci in range(CCH):
                                    nc.tensor.matmul(
                                        q_ps[:], lhsT=wq_s[cci][:, h * 128 : h * 128 + 128],
                                        rhs=x_s[cci][:],
                                        start=(cci == 0), stop=(cci == CCH - 1),
                                    )
                                norm_rope(q_ps, qhat[h][:, tsl], tci)

                            # V projection -> vnat (transposed to natural layout)
                            v_ps = psp.tile([128, TCH], F32, tag="xp", bufs=4, name="v_ps")
                            for cci in range(CCH):
                                nc.tensor.matmul(
                                    v_ps[:], lhsT=wv_s[cci][:], rhs=x_s[cci][:],
                                    start=(cci == 0), stop=(cci == CCH - 1),
                                )
                            v_sb = px.tile([128, TCH], F32, tag="vf", bufs=2, name="v_sb")
                            nc.any.tensor_copy(v_sb[:], v_ps[:])
                            for jj in range(4):
                                vt_ps = psp.tile([128, 128], F32, tag="vt", bufs=2, name="vt_ps")
                                nc.tensor.transpose(vt_ps[:], v_sb[:, jj * 128 : jj * 128 + 128], ident[:])
                                nc.vector.tensor_copy(vnat[4 * tci + jj][:], vt_ps[:])

                    # --------- attention + AG + out-proj pipeline ---------
                    with (
                        tc.tile_pool(name="pat", bufs=1) as pat,
                        tc.tile_pool(name="po", bufs=1) as po,
                        tc.tile_pool(name="aps", bufs=1, space="PSUM") as aps,
                    ):
                        mask_tri = pat.tile([128, 128], F32, name="mask_tri")
                        nc.sync.dma_start(mask_tri[:], masks[:])

                        # Wo channel slice for this rank (dynamic column offset)
                        pid = nc.sync.partition_id()
                        wo_off = (pid % TP) * QD
                        wo_s = []
                        for cci in range(CCH):
                            wo_t = po.tile([128, QD], F32R, name=f"wo{cci}")
                            nc.sync.dma_start(
                                wo_t[:],
                                woT[cci * 128 : cci * 128 + 128, bass.ds(wo_off, QD)].bitcast(F32R),
                            )
                            wo_s.append(wo_t)

                        for tqi in range(TC if PHASES >= 2 else 0):
                            tsl = slice(tqi * TCH, (tqi + 1) * TCH)
                            jmax = 4 * tqi + 4
                            for h in range(QH):
                                # one head per wave: with y/l bufs=2 the next
                                # head's accumulation always finds a free PSUM
                                # slot while this head's normalize chain drains
                                y_ps = aps.tile([128, TCH], F32, tag="y", bufs=2, name="y_ps")
                                l_ps = aps.tile([1, TCH], F32, tag="l", bufs=1, name="l_ps")
                                # diagonal blocks first: their longer
                                # mask->exp chains overlap the streaming full
                                # blocks instead of delaying the wave tail
                                j_order = list(range(4 * tqi, jmax)) + list(range(4 * tqi))
                                for ji, j in enumerate(j_order):
                                    off = max(0, (j - 4 * tqi) * 128)
                                    s_ps = aps.tile([128, TCH], F32, tag="s", bufs=3, name="s_ps")
                                    nc.tensor.matmul(
                                        s_ps[:, off:TCH],
                                        lhsT=khat[:, j * 128 : j * 128 + 128],
                                        rhs=qhat[h][:, tqi * TCH + off : (tqi + 1) * TCH],
                                        start=True,
                                        stop=True,
                                    )
                                    if j >= 4 * tqi:
                                        nc.vector.tensor_add(
                                            s_ps[:, off : off + 128],
                                            s_ps[:, off : off + 128],
                                            mask_tri[:],
                                        )
                                    p = pat.tile([128, TCH], F32R, tag="p", bufs=12, name="p")
                                    nc.scalar.activation(
                                        p[:, off:TCH], s_ps[:, off:TCH], AF.Exp, scale=SCALE
                                    )
                                    nc.tensor.matmul(
                                        l_ps[:, off:TCH],
                                        lhsT=ones_r[:, 0:1],
                                        rhs=p[:, off:TCH],
                                        start=(ji == 0),
                                        stop=(ji == jmax - 1),
                                    )
                                    nc.tensor.matmul(
                                        y_ps[:, off:TCH],
                                        lhsT=vnat[j][:],
                                        rhs=p[:, off:TCH],
                                        start=(ji == 0),
                                        stop=(ji == jmax - 1),
                                    )
                                rl = pat.tile([1, TCH], F32, tag="rl", bufs=2, name="rl")
                                nc.vector.reciprocal(rl[:], l_ps[:])
                                rb = pat.tile([128, TCH], F32, tag="rb", bufs=2, name="rb")
                                nc.gpsimd.partition_broadcast(rb[:], rl[:])
                                yh = pat.tile([128, TCH], F32, tag="yh", bufs=2, name="yh")
                                nc.vector.tensor_mul(yh[:], y_ps[:], rb[:])
                                nc.sync.dma_start(y_loc_t[tqi][h * 128 : h * 128 + 128, :], yh[:])

                            # AllGather this token chunk across the TP group
                            if NO_COLLECTIVE:
                                for q in range(TP):
                                    nc.sync.dma_start(
                                        y_all_t[tqi][q * QD : (q + 1) * QD, :], y_loc_t[tqi][:]
                                    )
                            else:
                                nc.gpsimd.collective_compute(
                                    "AllGather",
                                    mybir.AluOpType.bypass,
                                    replica_groups=[[0, 1, 2, 3], [4, 5, 6, 7]],
                                    ins=[y_loc_t[tqi][:]],
                                    outs=[y_all_t[tqi][:]],
                                )

                            # out-projection pass for this token chunk
                            if PHASES < 3:
                                continue
                            y_s = []
                            for cci in range(CCH):
                                y_t = po.tile([128, TCH], F32R, tag="yread", bufs=20, name=f"y{tqi}_{cci}")
                                nc.sync.dma_start(
                                    y_t[:], y_all_t[tqi][cci * 128 : cci * 128 + 128, :].bitcast(F32R)
                                )
                                y_s.append(y_t)
                            for jq in range(4):
                                o_ps = aps.tile([128, TCH], F32, tag="op", bufs=2, name="o_ps")
                                for cci in range(CCH):
                                    nc.tensor.matmul(
                                        o_ps[:], lhsT=wo_s[cci][:, jq * 128 : jq * 128 + 128],
                                        rhs=y_s[cci][:],
                                        start=(cci == 0), stop=(cci == CCH - 1),
                                    )
                                o_sb = po.tile([128, TCH], F32, tag="ob", bufs=3, name="o_sb")
                                nc.vector.tensor_copy(o_sb[:], o_ps[:])
                                nc.sync.dma_start(outT[jq * 128 : jq * 128 + 128, tsl], o_sb[:])

    nc.compile()
    return nc


def _get_nc():
    if "nc" not in _CACHE:
        _CACHE["nc"] = _build_nc()
    return _CACHE["nc"]


def _host_constants():
    if "consts" in _CACHE:
        return _CACHE["consts"]
    inv_freq = 1.0 / (ROPE_BASE ** (np.arange(0, HD, 2, dtype=np.float64) / HD))
    freqs = np.outer(np.arange(T, dtype=np.float64), inv_freq)  # [T, 64]
    cos = np.cos(freqs).astype(np.float32).T  # [64, T]
    sin = np.sin(freqs).astype(np.float32).T
    ccT = np.ascontiguousarray(np.concatenate([cos, cos], axis=0))   # [128, T]
    # the kernel computes swap(x*ss) (swap applied AFTER the multiply), so the
    # sin table is pre-swapped: swap(x)*[+sin;-sin] == swap(x*[-sin;+sin])
    ssT = np.ascontiguousarray(np.concatenate([-sin, sin], axis=0))  # [128, T]
    ii = np.arange(128, dtype=np.int64)[:, None]
    cc = np.arange(128, dtype=np.int64)[None, :]
    masks = np.where(cc >= ii, 0.0, NEG).astype(np.float32)
    ones = np.zeros((128, 128), dtype=np.float32)
    ones[:, 0] = 1.0
    _CACHE["consts"] = (ccT, ssT, masks, ones)
    return _CACHE["consts"]


def _in_maps(x, Wq, Wk, Wv, Wo):
    ccT, ssT, masks, ones = _host_constants()
    woT = np.ascontiguousarray(Wo.T.astype(np.float32))
    maps = []
    for c in range(NCORES):
        b, r = divmod(c, TP)
        maps.append(
            {
                "xT": np.ascontiguousarray(x[b].T.astype(np.float32)),
                "wqT": np.ascontiguousarray(Wq[r * QD : (r + 1) * QD, :].T.astype(np.float32)),
                "wkT": np.ascontiguousarray(Wk[r * HD : (r + 1) * HD, :].T.astype(np.float32)),
                "wvT": np.ascontiguousarray(Wv[r * HD : (r + 1) * HD, :].T.astype(np.float32)),
                "woT": woT,
                "ccT": ccT,
                "ssT": ssT,
                "masks": masks,
                "ones_in": ones,
            }
        )
    return maps


def _assemble(results):
    out = np.empty((B, T, C), dtype=np.float32)
    for c in range(NCORES):
        b, r = divmod(c, TP)
        out[b, :, r * QD : (r + 1) * QD] = results[c]["outT"].T
    return out


def kernel(x, Wq, Wk, Wv, Wo):
    from concourse.bass_utils import run_bass_kernel_spmd

    nc = _get_nc()
    maps = _in_maps(np.asarray(x), np.asarray(Wq), np.asarray(Wk), np.asarray(Wv), np.asarray(Wo))
    res = run_bass_kernel_spmd(nc, maps, list(range(NCORES)))
    return _assemble(res.results)



# revision 18
# speedup vs baseline: 5.4649x; 5.4649x over previous
"""Trainium2 Bass kernel for nn_Attention_28338194219036.

GQA attention block (QKV proj + QK-RMSNorm + RoPE + causal SDPA + out proj)
for x:[2,2048,2048], 16 q-heads / 4 kv-heads, head_dim 128.

Distribution over 8 NeuronCores: 2-way data parallel on batch x 4-way tensor
parallel on heads (core c: batch c//4, TP rank c%4 -> q-heads 4r..4r+3,
kv-head r). Per 512-token chunk a 4-rank AllGather exchanges head-shards of
y^T; each rank then computes its 512 output channels of Wo for that chunk.

v2 design (vs baseline):
- All matmul operands bf16 (same PE rate as f32r, half the DMA/SBUF bytes).
  PSUM accumulation stays f32.
- Host pre-lays every tensor in its exact SBUF layout ([128, N] with
  channel-blocks as column groups), so each load is ONE contiguous DMA.
  The baseline's 284 small DMAs serialized on ~650ns/descriptor HWDGE time.
- V is projected directly into its natural [token, head-dim] layout
  (x as lhsT), eliminating the PE transposes.
- Emission interleaves projection chunks, attention waves, AllGathers and
  out-projection chunks so the in-order PE never waits on a collective:
  proj0 proj1 attn0 AG0 proj2 attn1 AG1 op0 proj3 attn2 AG2 op1
  attn3a AG3a op2 attn3b AG3b op3a op3b.  The final 512-token chunk is
  split in two so the last AllGather hides behind out-projection PE work.
- Attention waves walk key blocks in ascending order (diagonal blocks
  last) so a fresh chunk's attention does not wait on its own RMSNorm/RoPE
  chain; the softmax denominator accumulates on the PE via a ones-column
  matmul; no max-subtraction is needed since QK-RMSNorm bounds scores.
"""

import os
import sys

for _p in ("/opt/trn_rl_repo", "/root/.axon_site/_ro/trn_rl_repo"):
    if os.path.isdir(_p) and _p not in sys.path:
        sys.path.append(_p)

import numpy as np

B, T, C = 2, 2048, 2048
NH, NKV, HD = 16, 4, 128
TP = 4            # tensor-parallel group size
NCORES = 8
QH = NH // TP     # q-heads per core (4)
QD = QH * HD      # q channels per core (512)
TC = 4            # projection token chunks of 512
TCH = T // TC     # 512
CCH = C // 128    # 16 channel chunks
ROPE_BASE = 10000.0
SCALE = 1.0 / float(np.sqrt(HD))
EPS = float(np.finfo(np.float32).eps)
NEG = -1.0e9
REPEAT = 1
NO_COLLECTIVE = False

# attention sub-chunks: (qoff, qlen); last projection chunk split in two so the
# final AllGather+readback hides behind out-projection PE work
SUBS = [(0, 512), (512, 512), (1024, 512), (1536, 256), (1792, 256)]

_CACHE = {}


def _build_nc():
    import concourse.mybir as mybir
    import concourse.tile as tile
    from concourse import bacc

    F32 = mybir.dt.float32
    BF16 = mybir.dt.bfloat16
    AF = mybir.ActivationFunctionType

    nc = bacc.Bacc("TRN2", target_bir_lowering=False, debug=False, num_devices=NCORES)

    x_in = nc.dram_tensor("x_in", [128, TC * CCH * TCH], BF16, kind="ExternalInput")
    wq_in = nc.dram_tensor("wq_in", [128, CCH * QD], BF16, kind="ExternalInput")
    wk_in = nc.dram_tensor("wk_in", [128, CCH * HD], BF16, kind="ExternalInput")
    wv_in = nc.dram_tensor("wv_in", [128, CCH * HD], BF16, kind="ExternalInput")
    wo_in = nc.dram_tensor("wo_in", [128, CCH * QD], BF16, kind="ExternalInput")
    cc_in = nc.dram_tensor("cc_in", [128, T], F32, kind="ExternalInput")
    ss_in = nc.dram_tensor("ss_in", [128, T], F32, kind="ExternalInput")
    mask_in = nc.dram_tensor("mask_in", [128, 128], F32, kind="ExternalInput")
    outT = nc.dram_tensor("outT", [QD, T], F32, kind="ExternalOutput")

    with tile.TileContext(nc) as tc:
        for _rep in range(REPEAT):
            with (
                tc.tile_pool(name="drp", bufs=1, space="DRAM") as drp,
                tc.tile_pool(name="pw", bufs=1) as pw,
                tc.tile_pool(name="px", bufs=1) as px,
                tc.tile_pool(name="pat", bufs=1) as pat,
                tc.tile_pool(name="psp", bufs=1, space="PSUM") as psp,
            ):
                y_loc = [drp.tile([QD, ql], BF16, name=f"y_loc{i}") for i, (_, ql) in enumerate(SUBS)]
                y_all = [drp.tile([C, ql], BF16, name=f"y_all{i}") for i, (_, ql) in enumerate(SUBS)]

                # ---- persistent SBUF state ----
                ones_b = pw.tile([128, 1], BF16, name="ones_b")
                nc.any.memset(ones_b[:], 1.0)
                epst = pw.tile([1, 1], F32, name="epst")
                nc.any.memset(epst[:], EPS)

                wk_s = pw.tile([128, CCH * HD], BF16, name="wk_s")
                nc.sync.dma_start(wk_s[:, : CCH * HD // 2], wk_in[:, : CCH * HD // 2])
                wq_s = pw.tile([128, CCH * QD], BF16, name="wq_s")
                wv_s = pw.tile([128, CCH * HD], BF16, name="wv_s")
                wo_s = pw.tile([128, CCH * QD], BF16, name="wo_s")
                mask_tri = pw.tile([128, 128], F32, name="mask_tri")

                qhat = [pw.tile([128, T], BF16, name=f"qhat{h}") for h in range(QH)]
                khat = pw.tile([128, T], BF16, name="khat")
                vnat = pw.tile([128, T], BF16, name="vnat")

                def load_x(tci, first=False):
                    """One x chunk -> SBUF [128, CCH*TCH] bf16, split in 4 DMAs
                    so the first projection matmuls start early."""
                    x_t = px.tile([128, CCH * TCH], BF16, tag="x", bufs=2, name=f"x{tci}")
                    base = tci * CCH * TCH
                    step = 4 * TCH
                    for i in range(4):
                        nc.sync.dma_start(
                            x_t[:, i * step : (i + 1) * step],
                            x_in[:, base + i * step : base + (i + 1) * step],
                        )
                    return x_t

                def load_tabs(tci):
                    tsl = slice(tci * TCH, (tci + 1) * TCH)
                    cc_t = px.tile([128, TCH], F32, tag="cc", bufs=2, name=f"cc{tci}")
                    nc.sync.dma_start(cc_t[:], cc_in[:, tsl])
                    ss_t = px.tile([128, TCH], F32, tag="ss", bufs=2, name=f"ss{tci}")
                    nc.sync.dma_start(ss_t[:], ss_in[:, tsl])
                    return cc_t, ss_t

                x_tiles = {0: load_x(0, first=True)}
                # remaining big loads, in the order the PE will need them
                nc.sync.dma_start(wk_s[:, CCH * HD // 2 :], wk_in[:, CCH * HD // 2 :])
                tab_tiles = {0: load_tabs(0)}
                nc.sync.dma_start(wv_s[:], wv_in[:])
                nc.sync.dma_start(wq_s[:], wq_in[:])
                nc.sync.dma_start(mask_tri[:], mask_in[:])
                nc.sync.dma_start(wo_s[:], wo_in[:])

                def norm_rope(x_ps, dest_slice, cc_t, ss_t):
                    """RMSNorm + RoPE a [128(hd), 512(tok)] psum chunk into
                    dest_slice (bf16 sbuf). The psum tile is consumed by two
                    quick act-engine reads (Square + Copy) so its bank frees
                    long before the DVE chain drains."""
                    sq = px.tile([128, TCH], BF16, tag="sq", bufs=2, name="sq")
                    nc.scalar.activation(sq[:], x_ps[:], AF.Square)
                    xs = px.tile([128, TCH], F32, tag="xs", bufs=2, name="xs")
                    nc.scalar.activation(xs[:], x_ps[:], AF.Copy)
                    msq = psp.tile([1, TCH], F32, tag="sm", bufs=2, name="msq")
                    nc.tensor.matmul(msq[:], lhsT=ones_b[:], rhs=sq[:], start=True, stop=True)
                    srt = px.tile([1, TCH], F32, tag="srt", bufs=2, name="srt")
                    nc.scalar.activation(srt[:], msq[:], AF.Sqrt, bias=epst[:], scale=1.0 / HD)
                    rin = px.tile([1, TCH], F32, tag="rin", bufs=2, name="rin")
                    nc.vector.reciprocal(rin[:], srt[:])
                    rbc = px.tile([128, TCH], F32, tag="rbc", bufs=2, name="rbc")
                    nc.gpsimd.partition_broadcast(rbc[:], rin[:])
                    # RoPE: xhat = x*cc + swap(x*ss_preswapped)
                    t1 = px.tile([128, TCH], F32, tag="t1", bufs=2, name="t1")
                    nc.vector.tensor_mul(t1[:], xs[:], ss_t[:])
                    t2 = px.tile([128, TCH], F32, tag="t2", bufs=2, name="t2")
                    nc.sync.dma_start(t2[0:64, :], t1[64:128, :])
                    nc.sync.dma_start(t2[64:128, :], t1[0:64, :])
                    u = px.tile([128, TCH], F32, tag="u", bufs=2, name="u")
                    nc.vector.tensor_mul(u[:], xs[:], cc_t[:])
                    nc.vector.tensor_add(u[:], u[:], t2[:])
                    nc.vector.tensor_mul(dest_slice, u[:], rbc[:])

                def proj(tci):
                    # prefetch next chunk's x + tables first
                    if tci + 1 < TC:
                        x_tiles[tci + 1] = load_x(tci + 1)
                        tab_tiles[tci + 1] = load_tabs(tci + 1)
                    x_t = x_tiles.pop(tci)
                    cc_t, ss_t = tab_tiles.pop(tci)
                    tsl = slice(tci * TCH, (tci + 1) * TCH)

                    # K projection
                    k_ps = psp.tile([128, TCH], F32, tag="G", bufs=4, name="k_ps")
                    for cci in range(CCH):
                        nc.tensor.matmul(
                            k_ps[:], lhsT=wk_s[:, cci * HD : (cci + 1) * HD],
                            rhs=x_t[:, cci * TCH : (cci + 1) * TCH],
                            start=(cci == 0), stop=(cci == CCH - 1),
                        )
                    norm_rope(k_ps, khat[:, tsl], cc_t, ss_t)

                    # V projection, directly in [token, hd] layout (x as lhsT)
                    v_ps = psp.tile([128, TCH], F32, tag="G", bufs=4, name="v_ps")
                    for jj in range(4):
                        for cci in range(CCH):
                            nc.tensor.matmul(
                                v_ps[:, jj * HD : (jj + 1) * HD],
                                lhsT=x_t[:, cci * TCH + jj * 128 : cci * TCH + (jj + 1) * 128],
                                rhs=wv_s[:, cci * HD : (cci + 1) * HD],
                                start=(cci == 0), stop=(cci == CCH - 1),
                            )
                    nc.any.tensor_copy(vnat[:, tsl], v_ps[:])

                    # Q projections
                    for h in range(QH):
                        q_ps = psp.tile([128, TCH], F32, tag="G", bufs=4, name="q_ps")
                        for cci in range(CCH):
                            nc.tensor.matmul(
                                q_ps[:], lhsT=wq_s[:, cci * QD + h * HD : cci * QD + (h + 1) * HD],
                                rhs=x_t[:, cci * TCH : (cci + 1) * TCH],
                                start=(cci == 0), stop=(cci == CCH - 1),
                            )
                        norm_rope(q_ps, qhat[h][:, tsl], cc_t, ss_t)

                def attn(si):
                    qoff, qlen = SUBS[si]
                    kb_tot = (qoff + qlen) // 128
                    dstart = qoff // 128  # first diagonal key block
                    yh = pat.tile([128, QH * TCH], BF16, tag="yh", bufs=2, name="yh")
                    for h in range(QH):
                        y_ps = psp.tile([128, TCH], F32, tag="y", bufs=2, name="y_ps")
                        l_ps = psp.tile([1, TCH], F32, tag="sm", bufs=2, name="l_ps")
                        for j in range(kb_tot):
                            off = max(0, j * 128 - qoff)
                            s_ps = psp.tile([128, TCH], F32, tag="G", bufs=4, name="s_ps")
                            nc.tensor.matmul(
                                s_ps[:, off:qlen],
                                lhsT=khat[:, j * 128 : (j + 1) * 128],
                                rhs=qhat[h][:, qoff + off : qoff + qlen],
                                start=True, stop=True,
                            )
                            if j >= dstart:
                                nc.vector.tensor_add(
                                    s_ps[:, off : off + 128],
                                    s_ps[:, off : off + 128],
                                    mask_tri[:],
                                )
                            p = pat.tile([128, TCH], BF16, tag="p", bufs=6, name="p")
                            nc.scalar.activation(p[:, off:qlen], s_ps[:, off:qlen], AF.Exp, scale=SCALE)
                            nc.tensor.matmul(
                                l_ps[:, off:qlen], lhsT=ones_b[:], rhs=p[:, off:qlen],
                                start=(j == 0), stop=(j == kb_tot - 1),
                            )
                            nc.tensor.matmul(
                                y_ps[:, off:qlen], lhsT=vnat[:, j * 128 : (j + 1) * 128],
                                rhs=p[:, off:qlen],
                                start=(j == 0), stop=(j == kb_tot - 1),
                            )
                        rl = pat.tile([1, TCH], F32, tag="rl", bufs=2, name="rl")
                        nc.vector.reciprocal(rl[:, :qlen], l_ps[:, :qlen])
                        rb = pat.tile([128, TCH], F32, tag="rb", bufs=2, name="rb")
                        nc.gpsimd.partition_broadcast(rb[:, :qlen], rl[:, :qlen])
                        nc.vector.tensor_mul(
                            yh[:, h * qlen : (h + 1) * qlen], y_ps[:, :qlen], rb[:, :qlen]
                        )
                    # ship the 4 head-shards to DRAM in one DMA
                    src = yh[:, : QH * qlen].rearrange("p (h t) -> p h t", h=QH)
                    dst = y_loc[si][:].rearrange("(h p) t -> p h t", p=128)
                    nc.sync.dma_start(dst, src)

                def allgather(si):
                    if NO_COLLECTIVE:
                        for q in range(TP):
                            nc.sync.dma_start(
                                y_all[si][q * QD : (q + 1) * QD, :], y_loc[si][:]
                            )
                    else:
                        import concourse.mybir as mybir

                        nc.gpsimd.collective_compute(
                            "AllGather",
                            mybir.AluOpType.bypass,
                            replica_groups=[[0, 1, 2, 3], [4, 5, 6, 7]],
                            ins=[y_loc[si][:]],
                            outs=[y_all[si][:]],
                        )

                def outproj(si):
                    qoff, qlen = SUBS[si]
                    yb = pat.tile([128, CCH * TCH], BF16, tag="yb", bufs=2, name="yb")
                    src = y_all[si][:].rearrange("(cci p) t -> p cci t", p=128)
                    dst = yb[:, : CCH * qlen].rearrange("p (cci t) -> p cci t", t=qlen)
                    nc.sync.dma_start(dst, src)
                    ob = pat.tile([128, 4 * TCH], F32, tag="ob", bufs=1, name="ob")
                    for jq in range(4):
                        o_ps = psp.tile([128, TCH], F32, tag="G", bufs=4, name="o_ps")
                        for cci in range(CCH):
                            nc.tensor.matmul(
                                o_ps[:, :qlen],
                                lhsT=wo_s[:, cci * QD + jq * 128 : cci * QD + (jq + 1) * 128],
                                rhs=yb[:, cci * qlen : (cci + 1) * qlen],
                                start=(cci == 0), stop=(cci == CCH - 1),
                            )
                        nc.vector.tensor_copy(ob[:, jq * qlen : (jq + 1) * qlen], o_ps[:, :qlen])
                    src = ob[:, : 4 * qlen].rearrange("p (jq t) -> p jq t", jq=4)
                    dst = outT[:, qoff : qoff + qlen].rearrange("(jq p) t -> p jq t", p=128)
                    nc.sync.dma_start(dst, src)

                # ---- emission schedule ----
                proj(0)
                proj(1)
                attn(0); allgather(0)
                proj(2)
                attn(1); allgather(1); outproj(0)
                proj(3)
                attn(2); allgather(2); outproj(1)
                attn(3); allgather(3); outproj(2)
                attn(4); allgather(4); outproj(3)
                outproj(4)

    nc.compile()
    return nc


def _get_nc():
    if "nc" not in _CACHE:
        _CACHE["nc"] = _build_nc()
    return _CACHE["nc"]


def _lay(wT):
    """[C, M] (already transposed weight) -> [128, (C/128)*M] with channel
    blocks as column groups: out[p, cci*M + j] = wT[cci*128 + p, j]."""
    Cd, M = wT.shape
    return np.ascontiguousarray(
        wT.reshape(Cd // 128, 128, M).transpose(1, 0, 2).reshape(128, -1)
    )


def _host_constants():
    if "consts" in _CACHE:
        return _CACHE["consts"]
    inv_freq = 1.0 / (ROPE_BASE ** (np.arange(0, HD, 2, dtype=np.float64) / HD))
    freqs = np.outer(np.arange(T, dtype=np.float64), inv_freq)  # [T, 64]
    cos = np.cos(freqs).astype(np.float32).T  # [64, T]
    sin = np.sin(freqs).astype(np.float32).T
    ccT = np.ascontiguousarray(np.concatenate([cos, cos], axis=0))   # [128, T]
    # the kernel computes swap(x*ss) (swap applied AFTER the multiply), so the
    # sin table is pre-swapped: swap(x)*[+sin;-sin] == swap(x*[-sin;+sin])
    ssT = np.ascontiguousarray(np.concatenate([-sin, sin], axis=0))  # [128, T]
    ii = np.arange(128, dtype=np.int64)[:, None]
    cc = np.arange(128, dtype=np.int64)[None, :]
    masks = np.where(cc >= ii, 0.0, NEG).astype(np.float32)
    _CACHE["consts"] = (ccT, ssT, masks)
    return _CACHE["consts"]


def _in_maps(x, Wq, Wk, Wv, Wo):
    import ml_dtypes

    BF = ml_dtypes.bfloat16
    ccT, ssT, masks = _host_constants()
    maps = []
    for c in range(NCORES):
        b, r = divmod(c, TP)
        xT = x[b].T.astype(BF)  # [C, T]
        x_l = np.concatenate(
            [_lay(xT[:, t * TCH : (t + 1) * TCH]) for t in range(TC)], axis=1
        )
        maps.append(
            {
                "x_in": np.ascontiguousarray(x_l),
                "wq_in": _lay(Wq[r * QD : (r + 1) * QD, :].T.astype(BF)),
                "wk_in": _lay(Wk[r * HD : (r + 1) * HD, :].T.astype(BF)),
                "wv_in": _lay(Wv[r * HD : (r + 1) * HD, :].T.astype(BF)),
                "wo_in": _lay(Wo[r * QD : (r + 1) * QD, :].T.astype(BF)),
                "cc_in": ccT,
                "ss_in": ssT,
                "mask_in": masks,
            }
        )
    return maps


def _assemble(results):
    out = np.empty((B, T, C), dtype=np.float32)
    for c in range(NCORES):
        b, r = divmod(c, TP)
        out[b, :, r * QD : (r + 1) * QD] = results[c]["outT"].T
    return out


def kernel(x, Wq, Wk, Wv, Wo):
    from concourse.bass_utils import run_bass_kernel_spmd

    nc = _get_nc()
    maps = _in_maps(np.asarray(x), np.asarray(Wq), np.asarray(Wk), np.asarray(Wv), np.asarray(Wo))
    res = run_bass_kernel_spmd(nc, maps, list(range(NCORES)))
    return _assemble(res.results)
